# revision 4
# baseline (speedup 1.0000x reference)
"""Sparse masked dot-product attention on 8 Trainium2 NeuronCores.

Problem: B=32, T=2048, D=128 attention with per-batch key-length masking
(valid_lens). out = softmax(mask(Q K^T / 256)) @ V, rows fully-masked -> 0.

Strategy:
  - Shard batches across 8 cores (4 per core), balanced by per-batch valid
    k-tile count (sparsity-aware: only k < valid_len contributes).
  - Per (batch, k-tile of 128): S^T = K_tile^T-stationary matmul vs Q^T
    (fp32r, full PE rate), exp on ScalarE (no max-subtraction needed:
    |scores| <= ~0.3), accumulate O'^T = sum_k V[k,:]^T P^T on PSUM, and
    accumulate P^T row-sums on VectorE for the softmax denominator.
  - Masking is done by host-side zero-padding K (masked scores -> exp(0)=1)
    and V (masked rows contribute 0 to O'), then subtracting the known
    constant overcount from the denominator.
  - Epilogue: per 128-wide q-tile, l via ones-matmul on the accumulated P
    sums, r = 1/(l - c), PE-transpose O'^T, scale by r, DMA out.
All cores run one SPMD program; per-slot k-tile counts are baked in at
build time from the actual valid_lens (max across cores per slot).
"""

import os
import sys
from contextlib import ExitStack

import numpy as np

for _p in ("/opt/trn_rl_repo", "/root/.axon_site/_ro/trn_rl_repo"):
    if os.path.isdir(_p) and _p not in sys.path:
        sys.path.insert(0, _p)

import concourse.bass as bass  # noqa: E402
import concourse.tile as tile  # noqa: E402
from concourse import bacc, mybir  # noqa: E402
from concourse.bass_utils import run_bass_kernel_spmd  # noqa: E402
from concourse.masks import make_identity  # noqa: E402

F32 = mybir.dt.float32
F32R = mybir.dt.float32r

B, T, D = 32, 2048, 128
N_CORES = 8
G = B // N_CORES  # batch slots per core
QH = 2  # q halves
QHW = T // QH  # 1024
INV_SCALE = 1.0 / 256.0  # reference: scores / (d / 0.5) = / 256

_program_cache: dict[tuple, tuple] = {}


def build_program(nkts: tuple[int, ...]):
    """Build the SPMD Bass program for per-slot k-tile counts `nkts`."""
    if nkts in _program_cache:
        return _program_cache[nkts]

    nkt_tot = sum(nkts)
    s_starts = np.concatenate([[0], np.cumsum(nkts)]).astype(int)

    nc = bacc.Bacc(
        "TRN2", target_bir_lowering=False, debug=False, num_devices=N_CORES
    )
    qt_ap = nc.dram_tensor("qt", [G, 128, T], F32R, kind="ExternalInput").ap()
    kts_ap = nc.dram_tensor("kts", [128, nkt_tot, 128], F32R, kind="ExternalInput").ap()
    vs_ap = nc.dram_tensor("vs", [128, nkt_tot, 128], F32R, kind="ExternalInput").ap()
    cs_ap = nc.dram_tensor("cs", [128, G], F32, kind="ExternalInput").ap()
    out_ap = nc.dram_tensor("out", [G, T, D], F32, kind="ExternalOutput").ap()

    with tile.TileContext(nc) as tc, ExitStack() as ctx:
        consts = ctx.enter_context(tc.tile_pool(name="consts", bufs=1))
        qtp = ctx.enter_context(tc.tile_pool(name="qtp", bufs=2))
        kvp = ctx.enter_context(tc.tile_pool(name="kvp", bufs=2))
        ptp = ctx.enter_context(tc.tile_pool(name="ptp", bufs=3))
        accp = ctx.enter_context(tc.tile_pool(name="accp", bufs=2))
        osbp = ctx.enter_context(tc.tile_pool(name="osbp", bufs=2))
        stgp = ctx.enter_context(tc.tile_pool(name="stgp", bufs=2))
        smallp = ctx.enter_context(tc.tile_pool(name="smallp", bufs=4))
        s_psp = ctx.enter_context(tc.tile_pool(name="s_ps", bufs=2, space="PSUM"))
        o_psp = ctx.enter_context(tc.tile_pool(name="o_ps", bufs=1, space="PSUM"))
        ep_psp = ctx.enter_context(tc.tile_pool(name="ep_ps", bufs=2, space="PSUM"))

        identity = consts.tile([128, 128], F32)
        make_identity(nc, identity)
        ones = consts.tile([128, 1], F32)
        nc.vector.memset(ones, 1.0)
        cs_sb = consts.tile([128, G], F32)
        nc.sync.dma_start(out=cs_sb, in_=cs_ap)

        for g in range(G):
            nkt = nkts[g]
            s0 = int(s_starts[g])
            qt_sb = qtp.tile([128, T], F32R, tag="qt")
            nc.sync.dma_start(out=qt_sb, in_=qt_ap[g])
            kt_sb = kvp.tile([128, nkt, 128], F32R, tag="kt")
            nc.sync.dma_start(out=kt_sb, in_=kts_ap[:, s0 : s0 + nkt, :])
            v_sb = kvp.tile([128, nkt, 128], F32R, tag="v")
            nc.sync.dma_start(out=v_sb, in_=vs_ap[:, s0 : s0 + nkt, :])

            for qh in range(QH):
                o_ps = o_psp.tile([128, QHW], F32, tag="o")
                acc = accp.tile([128, QHW], F32, tag="acc")
                for kt in range(nkt):
                    s_ps = s_psp.tile([128, QHW], F32, tag="s")
                    for c in range(QHW // 512):
                        nc.tensor.matmul(
                            s_ps[:, c * 512 : (c + 1) * 512],
                            lhsT=kt_sb[:, kt, :],
                            rhs=qt_sb[
                                :, qh * QHW + c * 512 : qh * QHW + (c + 1) * 512
                            ],
                            start=True,
                            stop=True,
                        )
                    pt = ptp.tile([128, QHW], F32R, tag="pt")
                    nc.scalar.activation(
                        out=pt,
                        in_=s_ps,
                        func=mybir.ActivationFunctionType.Exp,
                        scale=INV_SCALE,
                    )
                    for c in range(QHW // 512):
                        nc.tensor.matmul(
                            o_ps[:, c * 512 : (c + 1) * 512],
                            lhsT=v_sb[:, kt, :],
                            rhs=pt[:, c * 512 : (c + 1) * 512],
                            start=(kt == 0),
                            stop=(kt == nkt - 1),
                        )
                    if kt == 0:
                        nc.vector.tensor_copy(acc, pt)
                    else:
                        nc.vector.tensor_add(acc, acc, pt)

                # epilogue for this (g, qh): denominators, transpose, scale
                lt_ps = ep_psp.tile([128, 128], F32, tag="ep")
                for i in range(QHW // 128):
                    nc.tensor.matmul(
                        lt_ps[:, i : i + 1],
                        lhsT=acc[:, i * 128 : (i + 1) * 128],
                        rhs=ones,
                        start=True,
                        stop=True,
                    )
                lc = smallp.tile([128, QHW // 128], F32, tag="lc")
                nc.vector.tensor_scalar(
                    out=lc,
                    in0=lt_ps[:, 0 : QHW // 128],
                    scalar1=cs_sb[:, g : g + 1],
                    scalar2=None,
                    op0=mybir.AluOpType.subtract,
                )
                r = smallp.tile([128, QHW // 128], F32, tag="r")
                nc.vector.reciprocal(r, lc)

                o_sb = osbp.tile([128, QHW], F32, tag="osb")
                nc.vector.tensor_copy(o_sb, o_ps)
                stage = stgp.tile([128, QHW // 128, 128], F32, tag="stg")
                for i in range(QHW // 128):
                    tp_ps = ep_psp.tile([128, 128], F32, tag="ep")
                    nc.tensor.transpose(
                        tp_ps, o_sb[:, i * 128 : (i + 1) * 128], identity
                    )
                    nc.vector.tensor_scalar_mul(
                        stage[:, i, :], tp_ps, r[:, i : i + 1]
                    )
                nc.sync.dma_start(
                    out=out_ap[g, qh * QHW : (qh + 1) * QHW, :].rearrange(
                        "(t p) v -> p t v", p=128
                    ),
                    in_=stage,
                )
    nc.compile()
    _program_cache[nkts] = (nc, s_starts)
    return nc, s_starts


def prepare(queries, keys, values, valid_lens):
    """Host-side sharding: returns (nkts, s_starts-free in_maps, assign)."""
    queries = np.asarray(queries, dtype=np.float32)
    keys = np.asarray(keys, dtype=np.float32)
    values = np.asarray(values, dtype=np.float32)
    L = np.asarray(valid_lens).astype(np.int64)

    nkt_b = (L + 127) // 128  # valid k-tiles per batch (0..16)
    order = np.argsort(-nkt_b, kind="stable")
    # slot g holds ranks [8g, 8g+8); core c takes order[8g + c]
    assign = np.empty((N_CORES, G), dtype=np.int64)
    nkts = []
    for g in range(G):
        grp = order[N_CORES * g : N_CORES * (g + 1)]
        assign[:, g] = grp
        nkts.append(max(1, int(nkt_b[grp].max())))
    nkts = tuple(nkts)
    s_starts = np.concatenate([[0], np.cumsum(nkts)]).astype(int)
    nkt_tot = int(s_starts[-1])

    in_maps = []
    for core in range(N_CORES):
        qt_arr = np.empty((G, 128, T), dtype=np.float32)
        kts_arr = np.zeros((128, nkt_tot, 128), dtype=np.float32)
        vs_arr = np.zeros((128, nkt_tot, 128), dtype=np.float32)
        cs_arr = np.zeros((128, G), dtype=np.float32)
        for g in range(G):
            b = int(assign[core, g])
            Lb = int(L[b])
            nkt = nkts[g]
            s0 = int(s_starts[g])
            qt_arr[g] = queries[b].T
            kz = keys[b][: nkt * 128].copy()
            vz = values[b][: nkt * 128].copy()
            if Lb < nkt * 128:
                kz[Lb:] = 0.0
                vz[Lb:] = 0.0
            kts_arr[:, s0 : s0 + nkt, :] = kz.reshape(nkt, 128, 128).transpose(
                2, 0, 1
            )
            vs_arr[:, s0 : s0 + nkt, :] = vz.reshape(nkt, 128, 128).transpose(
                1, 0, 2
            )
            cs_arr[:, g] = float(nkt * 128 - Lb)
        in_maps.append(
            {"qt": qt_arr, "kts": kts_arr, "vs": vs_arr, "cs": cs_arr}
        )
    return nkts, in_maps, assign, L


def postprocess(results, assign, L):
    full = np.empty((B, T, D), dtype=np.float32)
    for core in range(N_CORES):
        out = results[core]["out"]
        for g in range(G):
            b = int(assign[core, g])
            if L[b] == 0:
                full[b] = 0.0
            else:
                full[b] = out[g]
    return full


def kernel(queries, keys, values, valid_lens):
    nkts, in_maps, assign, L = prepare(queries, keys, values, valid_lens)
    nc, _ = build_program(nkts)
    res = run_bass_kernel_spmd(nc, in_maps, list(range(N_CORES)))
    return postprocess(res.results, assign, L)
